# revision 1
# baseline (speedup 1.0000x reference)
"""Trainium2 Bass kernel for the SNN (LIF) network:

    cur1 = x.reshape(B,-1) @ W1.T + b1          (big fp32 matmul, once)
    200 sequential LIF steps on [B,1000] (layer 1), tiny matmul into 5
    outputs per step (layer 2), second LIF on [B,5].

Distribution over 8 cores:
  Phase A: contraction(K)-sharded exact-fp32 matmul -> per-core partial cur1
           [256, 1024(padded)], ReduceScatter(add) -> each core owns a
           32-row batch slice of cur1.
  Phase B: per-core LIF layer-1 scan over its 32-batch slice, hidden on
           partitions ([128, 8chunks x 32batch] tiles). One custom DVE
           instruction per step: mem' = beta*mem + cur - (mem > 1).
           ACT computes g = Sign(mem - 1) in bf16 (spk = (1+g)/2 folded
           into W2/b2 on the host).
  Phase C: every 4 steps, PE contracts g (stationary [128, 4*32]) against
           W2 chunks split hi/lo bf16 (exact), + bias matmul, into PSUM.
  Phase D: layer-2 LIF scan on [32, 5] per step; spk2 = (mem2 > 1) at the
           end. Outputs gathered on host.
"""
import sys

if "/opt/trn_rl_repo" not in sys.path:
    sys.path.insert(0, "/opt/trn_rl_repo")

import numpy as np
import ml_dtypes

# ---------------------------------------------------------------- constants
BETA = 0.95
T = 200
B = 256
NIN = 32000
NH = 1000
NO = 5

N_CORES = 8
KPAD = 32768           # NIN padded to 256*128
KC = KPAD // N_CORES   # 4096 contraction per core
KTILES = KC // 128     # 32
HPAD = 1024            # hidden padded
BLOC = B // N_CORES    # 32 batch rows per core after ReduceScatter
NCHUNK = HPAD // 128   # 8 hidden chunks of 128
G = 4                  # phase-C group size (steps per PE batch)
NGROUP = T // G        # 50
W1SCALE = 256.0        # W1 pre-scale so the fp16 lo-half stays normal

# ---------------------------------------------------------------- custom op
_LIF_NAME = "LIF_STEP_ANT"


def _register_lif_op():
    from concourse.dve_ops import (
        DveOp, OPS, CUSTOM_DVE_SPECS, _SUB_OPCODE_FOR_NAME, _CUSTOM_DVE_ROW_BASE,
    )
    from concourse.dve_spec import Spec, Src0, Src1, C0, One, lower as dve_lower, _has_src1
    from concourse.dve_uop import DveOpSpec

    for op in OPS:
        if op.name == _LIF_NAME:
            return op
    spec = Spec(
        body=Src0 * C0 + Src1 - (Src0 > One),
        reference=lambda in0, in1, s0: in0 * s0 + in1 - (in0 > 1.0).astype(np.float32),
    )
    if _LIF_NAME not in _SUB_OPCODE_FOR_NAME:
        _SUB_OPCODE_FOR_NAME[_LIF_NAME] = _CUSTOM_DVE_ROW_BASE + len(OPS)
    shas = {}
    for ver in ("v3", "v4"):
        s = DveOpSpec(
            name=_LIF_NAME,
            opcode=_SUB_OPCODE_FOR_NAME[_LIF_NAME],
            uops=dve_lower(spec, ver=ver),
            rd1_en=_has_src1(spec),
        )
        shas[ver] = s.sha(ver)
    op = DveOp(_LIF_NAME, spec, subdim=False, uops_sha=shas)
    OPS.append(op)
    CUSTOM_DVE_SPECS[_LIF_NAME] = op.spec
    return op


# ---------------------------------------------------------------- program
_PROGRAMS = {}  # sim -> (nc, lif_op)


def _build_program(sim=False):
    if sim in _PROGRAMS:
        return _PROGRAMS[sim]

    import concourse.bass as bass
    import concourse.tile as tile
    from concourse import bacc, mybir
    from concourse.masks import make_identity

    LIF = _register_lif_op()
    f32 = mybir.dt.float32
    bf16 = mybir.dt.bfloat16

    nc = bacc.Bacc("TRN2", target_bir_lowering=False, debug=False,
                   num_devices=1 if sim else N_CORES)

    f16 = mybir.dt.float16
    # inputs (per-core)
    xth_d = nc.dram_tensor("xth", [KTILES, 128, B], f16, kind="ExternalInput").ap()
    xtl_d = nc.dram_tensor("xtl", [KTILES, 128, B], f16, kind="ExternalInput").ap()
    w1h_d = nc.dram_tensor("w1h", [KTILES, 128, HPAD], f16, kind="ExternalInput").ap()
    w1l_d = nc.dram_tensor("w1l", [KTILES, 128, HPAD], f16, kind="ExternalInput").ap()
    b1c_d = nc.dram_tensor("b1c", [128, NCHUNK], f32, kind="ExternalInput").ap()
    w2hi_d = nc.dram_tensor("w2hi", [128, NCHUNK, NO], bf16, kind="ExternalInput").ap()
    w2lo_d = nc.dram_tensor("w2lo", [128, NCHUNK, NO], bf16, kind="ExternalInput").ap()
    b2e_d = nc.dram_tensor("b2e", [1, NO], f32, kind="ExternalInput").ap()
    # outputs (per-core batch slice), free layout = (t, o)
    mem2_d = nc.dram_tensor("mem2rec", [BLOC, T * NO], f32, kind="ExternalOutput").ap()
    spk2_d = nc.dram_tensor("spk2rec", [BLOC, T * NO], f32, kind="ExternalOutput").ap()
    curdbg_d = nc.dram_tensor("curdbg", [BLOC, HPAD], f32, kind="ExternalOutput").ap()

    with tile.TileContext(nc) as tc:
        with (
            tc.tile_pool(name="kin", bufs=3) as kpool,
            tc.tile_pool(name="win", bufs=3) as wpool,
            tc.tile_pool(name="psA", bufs=1, space="PSUM") as psA,
            tc.tile_pool(name="stage", bufs=1) as stage,
            tc.tile_pool(name="dram", bufs=1, space="DRAM") as dram,
            tc.tile_pool(name="mem", bufs=3) as mpool,
            tc.tile_pool(name="g4", bufs=3) as gpool,
            tc.tile_pool(name="psC", bufs=2, space="PSUM") as psC,
            tc.tile_pool(name="psT", bufs=2, space="PSUM") as psT,
        ):
            # ---------------- phase A: cur1 partial = xT_slice.T @ W1T_slice
            ps = [[psA.tile([128, 512], f32, tag=f"ps{mb}{nb}", name=f"ps{mb}{nb}")
                   for nb in range(2)] for mb in range(2)]
            for kt in range(KTILES):
                xh_t = kpool.tile([128, B], f16, tag="xth")
                nc.sync.dma_start(xh_t[:], xth_d[kt])
                xl_t = kpool.tile([128, B], f16, tag="xtl")
                nc.sync.dma_start(xl_t[:], xtl_d[kt])
                wh_t = wpool.tile([128, HPAD], f16, tag="w1h")
                nc.sync.dma_start(wh_t[:], w1h_d[kt])
                wl_t = wpool.tile([128, HPAD], f16, tag="w1l")
                nc.sync.dma_start(wl_t[:], w1l_d[kt])
                last = kt == KTILES - 1
                for mb in range(2):
                    xh_s = xh_t[:, mb * 128:(mb + 1) * 128]
                    xl_s = xl_t[:, mb * 128:(mb + 1) * 128]
                    # keep each stationary operand loaded across streams
                    for nb in range(2):
                        out = ps[mb][nb][:]
                        nc.tensor.matmul(out, xh_s, wl_t[:, nb * 512:(nb + 1) * 512],
                                         start=(kt == 0), stop=False)
                        nc.tensor.matmul(out, xh_s, wh_t[:, nb * 512:(nb + 1) * 512],
                                         start=False, stop=False)
                    for nb in range(2):
                        nc.tensor.matmul(ps[mb][nb][:], xl_s,
                                         wh_t[:, nb * 512:(nb + 1) * 512],
                                         start=False, stop=last)
            partial = dram.tile([B, HPAD], f32)
            for mb in range(2):
                cs = stage.tile([128, HPAD], f32, tag=f"curp{mb}")
                nc.scalar.activation(cs[:, 0:512], ps[mb][0][:],
                                     mybir.ActivationFunctionType.Copy, scale=1.0 / W1SCALE)
                nc.scalar.activation(cs[:, 512:1024], ps[mb][1][:],
                                     mybir.ActivationFunctionType.Copy, scale=1.0 / W1SCALE)
                nc.sync.dma_start(partial[mb * 128:(mb + 1) * 128, :], cs[:])

            # ---------------- ReduceScatter: each core gets its 32-batch slice
            rs_out = dram.tile([BLOC, HPAD], f32)
            if sim:
                # timing stand-in for the collective (single-core TimelineSim)
                nc.sync.dma_start(rs_out[:], partial[0:BLOC, :])
            else:
                nc.gpsimd.collective_compute(
                    "ReduceScatter",
                    mybir.AluOpType.add,
                    replica_groups=[list(range(N_CORES))],
                    ins=[partial.opt()],
                    outs=[rs_out.opt()],
                )

            # ---------------- transpose to scan layout + fold b1
            # curb[p, c*32 + b] = cur1[b, c*128 + p] + b1[c*128 + p]
            rsb = stage.tile([BLOC, HPAD], f32, tag="rsb")
            nc.sync.dma_start(rsb[:], rs_out[:])
            nc.sync.dma_start(curdbg_d[:], rsb[:])
            ident = stage.tile([BLOC, BLOC], f32, tag="ident")
            make_identity(nc, ident[:])
            b1t = stage.tile([128, NCHUNK], f32, tag="b1t")
            nc.sync.dma_start(b1t[:], b1c_d[:])
            curb = stage.tile([128, NCHUNK * BLOC], f32, tag="curb")
            for c in range(NCHUNK):
                pt = psT.tile([128, BLOC], f32, tag="pst")
                nc.tensor.transpose(pt[:], rsb[:, c * 128:(c + 1) * 128], ident[:])
                nc.scalar.activation(
                    curb[:, c * BLOC:(c + 1) * BLOC], pt[:],
                    mybir.ActivationFunctionType.Identity,
                    bias=b1t[:, c:c + 1], scale=1.0,
                )

            # ---------------- scan constants
            w2hi_t = stage.tile([128, NCHUNK, NO], bf16, tag="w2hi")
            nc.sync.dma_start(w2hi_t[:], w2hi_d[:])
            w2lo_t = stage.tile([128, NCHUNK, NO], bf16, tag="w2lo")
            nc.sync.dma_start(w2lo_t[:], w2lo_d[:])
            b2e_t = stage.tile([1, NO], f32, tag="b2e")
            nc.sync.dma_start(b2e_t[:], b2e_d[:])
            ones_t = stage.tile([1, 128], f32, tag="ones")
            nc.vector.memset(ones_t[:], 1.0)
            biasm1 = stage.tile([128, 1], f32, tag="bm1")
            nc.vector.memset(biasm1[:], -1.0)
            zeros_t = stage.tile([128, NCHUNK * BLOC], f32, tag="zeros")
            nc.vector.memset(zeros_t[:], 0.0)
            cur2buf = stage.tile([128, NGROUP * NO], f32, tag="cur2buf")

            # ---------------- phase B/C: layer-1 scan + layer-2 matmul
            mem_prev = zeros_t
            gt = None
            for t in range(1, T + 1):
                gi, sl = (t - 1) // G, (t - 1) % G
                if sl == 0:
                    gt = gpool.tile([128, NCHUNK, G * BLOC], bf16, tag="gt")
                m = mpool.tile([128, NCHUNK * BLOC], f32, tag="m")
                nc.vector._custom_dve(LIF, out=m[:], in0=mem_prev[:], in1=curb[:], s0=BETA)
                nc.scalar.activation(
                    gt[:, :, sl * BLOC:(sl + 1) * BLOC],
                    m[:].rearrange("p (c b) -> p c b", b=BLOC),
                    mybir.ActivationFunctionType.Sign, bias=biasm1[:], scale=1.0,
                )
                mem_prev = m
                if sl == G - 1:
                    pc = psC.tile([128, NO], f32, tag="psc")
                    for c in range(NCHUNK):
                        lhs = gt[:, c, :]
                        nc.tensor.matmul(pc[:], lhs, w2hi_t[:, c, :], start=(c == 0), stop=False)
                        nc.tensor.matmul(pc[:], lhs, w2lo_t[:, c, :], start=False, stop=False)
                    nc.tensor.matmul(pc[:], ones_t[:], b2e_t[:], start=False, stop=True)
                    nc.scalar.activation(
                        cur2buf[:, gi * NO:(gi + 1) * NO], pc[:],
                        mybir.ActivationFunctionType.Copy,
                    )

            # ---------------- rearrange cur2: [sl*32+b, gi*5+o] -> [b, t*5+o]
            cur2r = stage.tile([BLOC, T * NO], f32, tag="cur2r")
            cur2r_v = cur2r[:].rearrange("p (g s o) -> p g s o", s=G, o=NO)
            for sl in range(G):
                nc.sync.dma_start(
                    cur2r_v[:, :, sl, :],
                    cur2buf[sl * BLOC:(sl + 1) * BLOC, :].rearrange("p (g o) -> p g o", o=NO),
                )

            # ---------------- phase D: layer-2 scan
            mem2 = stage.tile([BLOC, T * NO], f32, tag="mem2")
            z32 = stage.tile([BLOC, NO], f32, tag="z32")
            nc.vector.memset(z32[:], 0.0)
            for t in range(T):
                in0 = z32[:] if t == 0 else mem2[:, (t - 1) * NO:t * NO]
                nc.vector._custom_dve(
                    LIF,
                    out=mem2[:, t * NO:(t + 1) * NO],
                    in0=in0,
                    in1=cur2r[:, t * NO:(t + 1) * NO],
                    s0=BETA,
                )
            spk2 = stage.tile([BLOC, T * NO], f32, tag="spk2")
            nc.vector.tensor_scalar(spk2[:], mem2[:], 1.0, None, mybir.AluOpType.is_gt)
            nc.sync.dma_start(mem2_d[:], mem2[:])
            nc.sync.dma_start(spk2_d[:], spk2[:])

    nc.compile()
    _PROGRAMS[sim] = (nc, LIF)
    return _PROGRAMS[sim]


# ---------------------------------------------------------------- host prep
def _prep_inputs(x, W1, b1, W2, b2):
    f32 = np.float32
    x_flat = np.ascontiguousarray(x.reshape(B, -1).astype(f32, copy=False))  # [256, 32000]
    xT = np.zeros((KPAD, B), f32)
    xT[:NIN] = x_flat.T
    xTh = xT.astype(np.float16)
    xTl = (xT - xTh.astype(f32)).astype(np.float16)
    w1T = np.zeros((KPAD, HPAD), f32)
    w1T[:NIN, :NH] = W1.astype(f32, copy=False).T * W1SCALE
    w1Th = w1T.astype(np.float16)
    w1Tl = (w1T - w1Th.astype(f32)).astype(np.float16)
    b1p = np.full(HPAD, -10.0, f32)
    b1p[:NH] = b1
    b1c = np.ascontiguousarray(b1p.reshape(NCHUNK, 128).T)          # [128, 8]
    W2e = np.zeros((HPAD, NO), f32)
    W2e[:NH] = 0.5 * W2.astype(f32, copy=False).T
    w2stack = np.ascontiguousarray(W2e.reshape(NCHUNK, 128, NO).transpose(1, 0, 2))  # [128,8,5]
    w2hi = w2stack.astype(ml_dtypes.bfloat16)
    w2lo = (w2stack - w2hi.astype(f32)).astype(ml_dtypes.bfloat16)
    b2e = (b2.astype(f32) + 0.5 * W2.astype(f32).sum(axis=1)).reshape(1, NO).astype(f32)

    in_maps = []
    for c in range(N_CORES):
        ksl = slice(c * KC, (c + 1) * KC)
        in_maps.append({
            "xth": np.ascontiguousarray(xTh[ksl]).reshape(KTILES, 128, B),
            "xtl": np.ascontiguousarray(xTl[ksl]).reshape(KTILES, 128, B),
            "w1h": np.ascontiguousarray(w1Th[ksl]).reshape(KTILES, 128, HPAD),
            "w1l": np.ascontiguousarray(w1Tl[ksl]).reshape(KTILES, 128, HPAD),
            "b1c": b1c,
            "w2hi": w2hi,
            "w2lo": w2lo,
            "b2e": b2e,
        })
    return in_maps


def _gather(results):
    spk_parts, mem_parts = [], []
    for r in results:
        mem_parts.append(r["mem2rec"].reshape(BLOC, T, NO).transpose(1, 0, 2))
        spk_parts.append(r["spk2rec"].reshape(BLOC, T, NO).transpose(1, 0, 2))
    mem2 = np.concatenate(mem_parts, axis=1).astype(np.float32)  # [200, 256, 5]
    spk2 = np.concatenate(spk_parts, axis=1).astype(np.float32)
    return spk2, mem2


def run_raw(inputs, **kwargs):
    """Build+run; returns BassKernelResults (for profiling from test.py)."""
    from concourse.bass_utils import run_bass_kernel_spmd

    nc, _ = _build_program()
    in_maps = _prep_inputs(**inputs)
    return run_bass_kernel_spmd(nc, in_maps, core_ids=list(range(N_CORES)), **kwargs)


def kernel(x, W1, b1, W2, b2):
    res = run_raw(dict(x=x, W1=W1, b1=b1, W2=W2, b2=b2))
    return _gather(res.results)


if __name__ == "__main__":
    rng = np.random.default_rng(0)
    ins = {
        "x": rng.standard_normal((B, 2, 80, 200)).astype(np.float32),
        "W1": rng.uniform(-1, 1, (NH, NIN)).astype(np.float32) / np.sqrt(NIN),
        "b1": rng.uniform(-1, 1, NH).astype(np.float32) / np.sqrt(NIN),
        "W2": rng.uniform(-1, 1, (NO, NH)).astype(np.float32) / np.sqrt(NH),
        "b2": rng.uniform(-1, 1, NO).astype(np.float32) / np.sqrt(NH),
    }
    spk2, mem2 = kernel(**ins)
    print("shapes:", spk2.shape, mem2.shape, spk2.dtype, mem2.dtype)
    print("spk2 mean:", spk2.mean(), "mem2 std:", mem2.std())



# revision 13
# speedup vs baseline: 1.1870x; 1.1870x over previous
"""Trainium2 Bass kernel for the SNN (LIF) network:

    cur1 = x.reshape(B,-1) @ W1.T + b1          (big fp32 matmul, once)
    200 sequential LIF steps on [B,1000] (layer 1), tiny matmul into 5
    outputs per step (layer 2), second LIF on [B,5].

Distribution over 8 cores (v2):
  Phase A: contraction(K)-sharded exact-fp32 matmul (fp16 hi/lo split, 3
           products) split into TWO hidden halves; the ReduceScatter of
           half 0 is triggered while the PE computes half 1, hiding most
           of the collective. x tiles stay resident in SBUF.
  Phase B: per-core LIF layer-1 scan over its 32-batch slice, hidden on
           partitions ([128, 8chunks x 32batch] tiles). One custom DVE
           instruction per step: mem' = beta*mem + cur - (mem > 1).
           m tiles are grouped 4 steps per tile so ACT computes
           g = Sign(mem - 1) for 4 steps in ONE instruction.
  Phase C: every 4 steps, PE contracts g (stationary [128, 4*32]) against
           W2 chunks split hi/lo bf16 (exact), + bias matmul, into PSUM.
  Phase D: layer-2 LIF interleaved into the scan with a 2-group lag
           (no serial tail). Outputs gathered on host.
"""
import sys

if "/opt/trn_rl_repo" not in sys.path:
    sys.path.insert(0, "/opt/trn_rl_repo")

import numpy as np
import ml_dtypes

# ---------------------------------------------------------------- constants
BETA = 0.95
T = 200
B = 256
NIN = 32000
NH = 1000
NO = 5

N_CORES = 8
KPAD = 32768           # NIN padded to 256*128
KC = KPAD // N_CORES   # 4096 contraction per core
KTILES = KC // 128     # 32
HPAD = 1024            # hidden padded
HHALF = HPAD // 2      # 512 per phase-A half
BLOC = B // N_CORES    # 32 batch rows per core after ReduceScatter
NCHUNK = HPAD // 128   # 8 hidden chunks of 128
G = 4                  # phase-C group size (steps per PE batch)
NGROUP = T // G        # 50
NG_LAG = 2             # phase-D trails phase-C by this many groups
W1SCALE = 256.0        # W1 pre-scale so the fp16 lo-half stays normal
DEBUG = False          # emit curb/cur2 debug outputs (dbg_check.py)

# ---------------------------------------------------------------- custom op
_LIF_NAME = "LIF_STEP_ANT"


def _register_lif_op():
    from concourse.dve_ops import (
        DveOp, OPS, CUSTOM_DVE_SPECS, _SUB_OPCODE_FOR_NAME, _CUSTOM_DVE_ROW_BASE,
    )
    from concourse.dve_spec import Spec, Src0, Src1, C0, One, lower as dve_lower, _has_src1
    from concourse.dve_uop import DveOpSpec

    for op in OPS:
        if op.name == _LIF_NAME:
            return op
    spec = Spec(
        body=Src0 * C0 + Src1 - (Src0 > One),
        reference=lambda in0, in1, s0: in0 * s0 + in1 - (in0 > 1.0).astype(np.float32),
    )
    if _LIF_NAME not in _SUB_OPCODE_FOR_NAME:
        _SUB_OPCODE_FOR_NAME[_LIF_NAME] = _CUSTOM_DVE_ROW_BASE + len(OPS)
    shas = {}
    for ver in ("v3", "v4"):
        s = DveOpSpec(
            name=_LIF_NAME,
            opcode=_SUB_OPCODE_FOR_NAME[_LIF_NAME],
            uops=dve_lower(spec, ver=ver),
            rd1_en=_has_src1(spec),
        )
        shas[ver] = s.sha(ver)
    op = DveOp(_LIF_NAME, spec, subdim=False, uops_sha=shas)
    OPS.append(op)
    CUSTOM_DVE_SPECS[_LIF_NAME] = op.spec
    return op


# ---------------------------------------------------------------- program
_PROGRAMS = {}  # sim -> (nc, lif_op)


def _build_program(sim=False):
    if sim in _PROGRAMS:
        return _PROGRAMS[sim]

    import concourse.bass as bass
    import concourse.tile as tile
    from concourse import bacc, mybir
    from concourse.masks import make_identity

    LIF = _register_lif_op()
    f32 = mybir.dt.float32
    bf16 = mybir.dt.bfloat16
    f16 = mybir.dt.float16

    nc = bacc.Bacc("TRN2", target_bir_lowering=False, debug=False,
                   num_devices=1 if sim else N_CORES)

    # inputs (per-core).  x/W stored partition-major: free dims (kt, cols)
    xth_d = nc.dram_tensor("xth", [128, KTILES, B], f16, kind="ExternalInput").ap()
    xtl_d = nc.dram_tensor("xtl", [128, KTILES, B], f16, kind="ExternalInput").ap()
    w1h_d = nc.dram_tensor("w1h", [128, KTILES, HPAD], f16, kind="ExternalInput").ap()
    w1l_d = nc.dram_tensor("w1l", [128, KTILES, HPAD], f16, kind="ExternalInput").ap()
    b1c_d = nc.dram_tensor("b1c", [128, NCHUNK], f32, kind="ExternalInput").ap()
    w2f_d = nc.dram_tensor("w2f", [128, NCHUNK, NO], f16, kind="ExternalInput").ap()
    b2e_d = nc.dram_tensor("b2e", [1, NO], f32, kind="ExternalInput").ap()
    # outputs (per-core batch slice), free layout = (t, o)
    mem2_d = nc.dram_tensor("mem2rec", [BLOC, T * NO], f32, kind="ExternalOutput").ap()
    spk2_d = nc.dram_tensor("spk2rec", [BLOC, T * NO], f32, kind="ExternalOutput").ap()
    if DEBUG:
        curb_dbg = nc.dram_tensor("curbdbg", [128, NCHUNK * BLOC], f32, kind="ExternalOutput").ap()
        cur2_dbg = nc.dram_tensor("cur2dbg", [128, NGROUP * NO], f32, kind="ExternalOutput").ap()

    with tile.TileContext(nc) as tc:
        with (
            tc.tile_pool(name="xres", bufs=1) as xres,
            tc.tile_pool(name="win", bufs=3) as wpool,
            tc.tile_pool(name="psA", bufs=2, space="PSUM") as psA,
            tc.tile_pool(name="stage", bufs=1) as stage,
            tc.tile_pool(name="dram", bufs=1, space="DRAM") as dram,
            tc.tile_pool(name="mem", bufs=3) as mpool,
            tc.tile_pool(name="g4", bufs=3) as gpool,
            tc.tile_pool(name="psC", bufs=2, space="PSUM") as psC,
            tc.tile_pool(name="psT", bufs=2, space="PSUM") as psT,
        ):
            # ---------------- x resident tiles + small constants up front
            xh_res = xres.tile([128, KTILES, B], f16, tag="xh")
            xl_res = xres.tile([128, KTILES, B], f16, tag="xl")
            NXD = 8  # x DMAs per tensor
            kstep = KTILES // NXD
            for i in range(NXD):
                sl = slice(i * kstep, (i + 1) * kstep)
                nc.sync.dma_start(xh_res[:, sl, :], xth_d[:, sl, :])
            for i in range(NXD):
                sl = slice(i * kstep, (i + 1) * kstep)
                nc.sync.dma_start(xl_res[:, sl, :], xtl_d[:, sl, :])

            b1t = stage.tile([128, NCHUNK], f32, tag="b1t")
            nc.sync.dma_start(b1t[:], b1c_d[:])
            w2f_t = stage.tile([128, NCHUNK, NO], f16, tag="w2f")
            nc.sync.dma_start(w2f_t[:], w2f_d[:])
            b2e_t = stage.tile([1, NO], f32, tag="b2e")
            nc.sync.dma_start(b2e_t[:], b2e_d[:])

            ident = stage.tile([BLOC, BLOC], f32, tag="ident")
            make_identity(nc, ident[:])
            ones_t = stage.tile([1, 128], f32, tag="ones")
            nc.vector.memset(ones_t[:], 1.0)
            biasm1 = stage.tile([128, 1], f32, tag="bm1")
            nc.vector.memset(biasm1[:], -1.0)
            zeros_t = stage.tile([128, NCHUNK * BLOC], f32, tag="zeros")
            nc.vector.memset(zeros_t[:], 0.0)
            z32 = stage.tile([BLOC, NO], f32, tag="z32")
            nc.vector.memset(z32[:], 0.0)

            # ---------------- phase A: two hidden halves, RS pipelined
            KG = 4  # kt per W-tile
            partials = []
            rs_outs = []
            for h in range(2):
                hs = slice(h * HHALF, (h + 1) * HHALF)
                ps = [psA.tile([128, HHALF], f32, tag=f"psA{mb}", name=f"ps{h}{mb}")
                      for mb in range(2)]
                for kg in range(KTILES // KG):
                    wl_t = wpool.tile([128, KG, HHALF], f16, tag="w1l")
                    nc.sync.dma_start(wl_t[:], w1l_d[:, kg * KG:(kg + 1) * KG, hs])
                    wh_t = wpool.tile([128, KG, HHALF], f16, tag="w1h")
                    nc.sync.dma_start(wh_t[:], w1h_d[:, kg * KG:(kg + 1) * KG, hs])
                    for i in range(KG):
                        kt = kg * KG + i
                        first = kt == 0
                        last = kt == KTILES - 1
                        for mb in range(2):
                            bs = slice(mb * 128, (mb + 1) * 128)
                            xh_s = xh_res[:, kt, bs]
                            xl_s = xl_res[:, kt, bs]
                            out = ps[mb][:]
                            nc.tensor.matmul(out, xh_s, wl_t[:, i, :],
                                             start=first, stop=False)
                            nc.tensor.matmul(out, xh_s, wh_t[:, i, :],
                                             start=False, stop=False)
                            nc.tensor.matmul(out, xl_s, wh_t[:, i, :],
                                             start=False, stop=last)
                partial = dram.tile([B, HHALF], f32, tag=f"partial{h}")
                for mb in range(2):
                    cs = stage.tile([128, HHALF], f32, tag=f"curp{mb}", name=f"cs{h}{mb}")
                    nc.scalar.activation(cs[:], ps[mb][:],
                                         mybir.ActivationFunctionType.Copy,
                                         scale=1.0 / W1SCALE)
                    nc.sync.dma_start(partial[mb * 128:(mb + 1) * 128, :], cs[:])
                rs_out = dram.tile([BLOC, HHALF], f32, tag=f"rsout{h}")
                if sim:
                    nc.sync.dma_start(rs_out[:], partial[0:BLOC, :])
                else:
                    nc.gpsimd.collective_compute(
                        "ReduceScatter",
                        mybir.AluOpType.add,
                        replica_groups=[list(range(N_CORES))],
                        ins=[partial.opt()],
                        outs=[rs_out.opt()],
                    )
                partials.append(partial)
                rs_outs.append(rs_out)

            # ---------------- transpose to scan layout + fold b1
            # curb[p, c*32 + b] = cur1[b, c*128 + p] + b1[c*128 + p]
            curb = stage.tile([128, NCHUNK * BLOC], f32, tag="curb")
            for h in range(2):
                rsb = stage.tile([BLOC, HHALF], f32, tag="rsb", name=f"rsb{h}")
                nc.sync.dma_start(rsb[:], rs_outs[h][:])
                for cc in range(NCHUNK // 2):
                    c = h * (NCHUNK // 2) + cc
                    pt = psT.tile([128, BLOC], f32, tag="pst")
                    nc.tensor.transpose(pt[:], rsb[:, cc * 128:(cc + 1) * 128], ident[:])
                    nc.scalar.activation(
                        curb[:, c * BLOC:(c + 1) * BLOC], pt[:],
                        mybir.ActivationFunctionType.Identity,
                        bias=b1t[:, c:c + 1], scale=1.0,
                    )

            # ---------------- phase B/C/D: fused scan
            cur2buf = stage.tile([128, NGROUP * NO], f32, tag="cur2buf")
            # batch-major layer-2 buffers: [32, (g, s, o)] == output layout
            cur2r = stage.tile([BLOC, T * NO], f32, tag="cur2r")
            cur2r_v = cur2r[:].rearrange("p (g s o) -> p g s o", s=G, o=NO)
            mem2r = stage.tile([BLOC, T * NO], f32, tag="mem2r")

            def fold_and_phase_d(g0, g1):
                """Move cur2buf groups [g0,g1) to batch-major cur2r, then run
                the layer-2 LIF steps for those groups."""
                for sl in range(G):
                    nc.sync.dma_start(
                        cur2r_v[:, g0:g1, sl, :],
                        cur2buf[sl * BLOC:(sl + 1) * BLOC,
                                g0 * NO:g1 * NO].rearrange("p (g o) -> p g o", o=NO),
                    )
                for t2 in range(g0 * G, g1 * G):
                    in0 = z32[:] if t2 == 0 else mem2r[:, (t2 - 1) * NO:t2 * NO]
                    nc.vector._custom_dve(
                        LIF,
                        out=mem2r[:, t2 * NO:(t2 + 1) * NO],
                        in0=in0,
                        in1=cur2r[:, t2 * NO:(t2 + 1) * NO],
                        s0=BETA,
                    )

            SB = 10  # groups per phase-D superbatch
            mg_prev = None
            for gi in range(NGROUP):
                mg = mpool.tile([128, G, NCHUNK * BLOC], f32, tag="mg")
                for sl in range(G):
                    if gi == 0 and sl == 0:
                        in0 = zeros_t[:]
                    elif sl == 0:
                        in0 = mg_prev[:, G - 1, :]
                    else:
                        in0 = mg[:, sl - 1, :]
                    nc.vector._custom_dve(LIF, out=mg[:, sl, :], in0=in0,
                                          in1=curb[:], s0=BETA)
                mg_prev = mg
                # batched sign for the whole group: gt[p, c, s*32+b] = sign(m-1)
                gt = gpool.tile([128, NCHUNK, G * BLOC], f16, tag="gt")
                nc.scalar.activation(
                    gt[:].rearrange("p c (s b) -> p s c b", b=BLOC),
                    mg[:].rearrange("p s (c b) -> p s c b", b=BLOC),
                    mybir.ActivationFunctionType.Sign, bias=biasm1[:], scale=1.0,
                )
                # phase C: cur2 for the group (single fp16 product per chunk)
                pc = psC.tile([128, NO], f32, tag="psc")
                for c in range(NCHUNK):
                    nc.tensor.matmul(pc[:], gt[:, c, :], w2f_t[:, c, :],
                                     start=(c == 0), stop=False)
                nc.tensor.matmul(pc[:], ones_t[:], b2e_t[:], start=False, stop=True)
                nc.scalar.activation(
                    cur2buf[:, gi * NO:(gi + 1) * NO], pc[:],
                    mybir.ActivationFunctionType.Copy,
                )
                if (gi + 1) % SB == 0:
                    fold_and_phase_d(gi + 1 - SB, gi + 1)
            if DEBUG:
                nc.sync.dma_start(curb_dbg[:], curb[:])
                nc.sync.dma_start(cur2_dbg[:], cur2buf[:])

            spk2 = stage.tile([BLOC, T * NO], f32, tag="spk2")
            nc.vector.tensor_scalar(spk2[:], mem2r[:], 1.0, None, mybir.AluOpType.is_gt)
            nc.sync.dma_start(mem2_d[:], mem2r[:])
            nc.sync.dma_start(spk2_d[:], spk2[:])

    nc.compile()
    _PROGRAMS[sim] = (nc, LIF)
    return _PROGRAMS[sim]


# ---------------------------------------------------------------- host prep
def _prep_inputs(x, W1, b1, W2, b2):
    f32 = np.float32
    x_flat = np.ascontiguousarray(x.reshape(B, -1).astype(f32, copy=False))  # [256, 32000]
    xT = np.zeros((KPAD, B), f32)
    xT[:NIN] = x_flat.T
    xTh = xT.astype(np.float16)
    xTl = (xT - xTh.astype(f32)).astype(np.float16)
    w1T = np.zeros((KPAD, HPAD), f32)
    w1T[:NIN, :NH] = W1.astype(f32, copy=False).T * W1SCALE
    w1Th = w1T.astype(np.float16)
    w1Tl = (w1T - w1Th.astype(f32)).astype(np.float16)
    b1p = np.full(HPAD, -10.0, f32)
    b1p[:NH] = b1
    b1c = np.ascontiguousarray(b1p.reshape(NCHUNK, 128).T)          # [128, 8]
    W2e = np.zeros((HPAD, NO), f32)
    W2e[:NH] = 0.5 * W2.astype(f32, copy=False).T
    w2stack = np.ascontiguousarray(W2e.reshape(NCHUNK, 128, NO).transpose(1, 0, 2))  # [128,8,5]
    w2f = w2stack.astype(np.float16)
    b2e = (b2.astype(f32) + 0.5 * W2.astype(f32).sum(axis=1)).reshape(1, NO).astype(f32)

    in_maps = []
    for c in range(N_CORES):
        ksl = slice(c * KC, (c + 1) * KC)
        # partition-major relayouts: [128, KTILES, cols]
        xh_r = np.ascontiguousarray(xTh[ksl].reshape(KTILES, 128, B).transpose(1, 0, 2))
        xl_r = np.ascontiguousarray(xTl[ksl].reshape(KTILES, 128, B).transpose(1, 0, 2))
        wh_r = np.ascontiguousarray(w1Th[ksl].reshape(KTILES, 128, HPAD).transpose(1, 0, 2))
        wl_r = np.ascontiguousarray(w1Tl[ksl].reshape(KTILES, 128, HPAD).transpose(1, 0, 2))
        in_maps.append({
            "xth": xh_r,
            "xtl": xl_r,
            "w1h": wh_r,
            "w1l": wl_r,
            "b1c": b1c,
            "w2f": w2f,
            "b2e": b2e,
        })
    return in_maps


def _gather(results):
    spk_parts, mem_parts = [], []
    for r in results:
        mem_parts.append(r["mem2rec"].reshape(BLOC, T, NO).transpose(1, 0, 2))
        spk_parts.append(r["spk2rec"].reshape(BLOC, T, NO).transpose(1, 0, 2))
    mem2 = np.concatenate(mem_parts, axis=1).astype(np.float32)  # [200, 256, 5]
    spk2 = np.concatenate(spk_parts, axis=1).astype(np.float32)
    return spk2, mem2


def run_raw(inputs, **kwargs):
    """Build+run; returns BassKernelResults (for profiling from test.py)."""
    from concourse.bass_utils import run_bass_kernel_spmd

    nc, _ = _build_program()
    in_maps = _prep_inputs(**inputs)
    return run_bass_kernel_spmd(nc, in_maps, core_ids=list(range(N_CORES)), **kwargs)


def kernel(x, W1, b1, W2, b2):
    res = run_raw(dict(x=x, W1=W1, b1=b1, W2=W2, b2=b2))
    return _gather(res.results)


if __name__ == "__main__":
    rng = np.random.default_rng(0)
    ins = {
        "x": rng.standard_normal((B, 2, 80, 200)).astype(np.float32),
        "W1": rng.uniform(-1, 1, (NH, NIN)).astype(np.float32) / np.sqrt(NIN),
        "b1": rng.uniform(-1, 1, NH).astype(np.float32) / np.sqrt(NIN),
        "W2": rng.uniform(-1, 1, (NO, NH)).astype(np.float32) / np.sqrt(NH),
        "b2": rng.uniform(-1, 1, NO).astype(np.float32) / np.sqrt(NH),
    }
    spk2, mem2 = kernel(**ins)
    print("shapes:", spk2.shape, mem2.shape, spk2.dtype, mem2.dtype)
    print("spk2 mean:", spk2.mean(), "mem2 std:", mem2.std())


# revision 18
# speedup vs baseline: 1.1974x; 1.0088x over previous
"""Trainium2 Bass kernel for the SNN (LIF) network:

    cur1 = x.reshape(B,-1) @ W1.T + b1          (big fp32 matmul, once)
    200 sequential LIF steps on [B,1000] (layer 1), tiny matmul into 5
    outputs per step (layer 2), second LIF on [B,5].

Distribution over 8 cores (v2):
  Phase A: contraction(K)-sharded exact-fp32 matmul (fp16 hi/lo split, 3
           products) split into TWO hidden halves; the ReduceScatter of
           half 0 is triggered while the PE computes half 1, hiding most
           of the collective. x tiles stay resident in SBUF.
  Phase B: per-core LIF layer-1 scan over its 32-batch slice, hidden on
           partitions ([128, 8chunks x 32batch] tiles). One custom DVE
           instruction per step: mem' = beta*mem + cur - (mem > 1).
           m tiles are grouped 4 steps per tile so ACT computes
           g = Sign(mem - 1) for 4 steps in ONE instruction.
  Phase C: every 4 steps, PE contracts g (stationary [128, 4*32]) against
           W2 chunks split hi/lo bf16 (exact), + bias matmul, into PSUM.
  Phase D: layer-2 LIF interleaved into the scan with a 2-group lag
           (no serial tail). Outputs gathered on host.
"""
import sys

if "/opt/trn_rl_repo" not in sys.path:
    sys.path.insert(0, "/opt/trn_rl_repo")

import numpy as np
import ml_dtypes

# ---------------------------------------------------------------- constants
BETA = 0.95
T = 200
B = 256
NIN = 32000
NH = 1000
NO = 5

N_CORES = 8
KPAD = 32768           # NIN padded to 256*128
KC = KPAD // N_CORES   # 4096 contraction per core
KTILES = KC // 128     # 32
HPAD = 1024            # hidden padded
HHALF = HPAD // 2      # 512 per phase-A half
BLOC = B // N_CORES    # 32 batch rows per core after ReduceScatter
NCHUNK = HPAD // 128   # 8 hidden chunks of 128
G = 4                  # phase-C group size (steps per PE batch)
NGROUP = T // G        # 50
NG_LAG = 2             # phase-D trails phase-C by this many groups
W1SCALE = 256.0        # W1 pre-scale so the fp16 lo-half stays normal
DEBUG = False          # emit curb/cur2 debug outputs (dbg_check.py)

# ---------------------------------------------------------------- custom op
_LIF_NAME = "LIF_STEP_ANT"


def _register_lif_op():
    from concourse.dve_ops import (
        DveOp, OPS, CUSTOM_DVE_SPECS, _SUB_OPCODE_FOR_NAME, _CUSTOM_DVE_ROW_BASE,
    )
    from concourse.dve_spec import Spec, Src0, Src1, C0, One, lower as dve_lower, _has_src1
    from concourse.dve_uop import DveOpSpec

    for op in OPS:
        if op.name == _LIF_NAME:
            return op
    spec = Spec(
        body=Src0 * C0 + Src1 - (Src0 > One),
        reference=lambda in0, in1, s0: in0 * s0 + in1 - (in0 > 1.0).astype(np.float32),
    )
    if _LIF_NAME not in _SUB_OPCODE_FOR_NAME:
        _SUB_OPCODE_FOR_NAME[_LIF_NAME] = _CUSTOM_DVE_ROW_BASE + len(OPS)
    shas = {}
    for ver in ("v3", "v4"):
        s = DveOpSpec(
            name=_LIF_NAME,
            opcode=_SUB_OPCODE_FOR_NAME[_LIF_NAME],
            uops=dve_lower(spec, ver=ver),
            rd1_en=_has_src1(spec),
        )
        shas[ver] = s.sha(ver)
    op = DveOp(_LIF_NAME, spec, subdim=False, uops_sha=shas)
    OPS.append(op)
    CUSTOM_DVE_SPECS[_LIF_NAME] = op.spec
    return op


# ---------------------------------------------------------------- program
_PROGRAMS = {}  # sim -> (nc, lif_op)


def _build_program(sim=False):
    if sim in _PROGRAMS:
        return _PROGRAMS[sim]

    import concourse.bass as bass
    import concourse.tile as tile
    from concourse import bacc, mybir
    from concourse.masks import make_identity

    LIF = _register_lif_op()
    f32 = mybir.dt.float32
    bf16 = mybir.dt.bfloat16
    f16 = mybir.dt.float16

    nc = bacc.Bacc("TRN2", target_bir_lowering=False, debug=False,
                   num_devices=1 if sim else N_CORES)

    # inputs (per-core).  x/W stored partition-major: free dims (kt, cols)
    xth_d = nc.dram_tensor("xth", [128, KTILES, B], f16, kind="ExternalInput").ap()
    xtl_d = nc.dram_tensor("xtl", [128, KTILES, B], f16, kind="ExternalInput").ap()
    w1h_d = nc.dram_tensor("w1h", [128, KTILES, HPAD], f16, kind="ExternalInput").ap()
    w1l_d = nc.dram_tensor("w1l", [128, KTILES, HPAD], f16, kind="ExternalInput").ap()
    b1c_d = nc.dram_tensor("b1c", [128, NCHUNK], f32, kind="ExternalInput").ap()
    w2f_d = nc.dram_tensor("w2f", [128, NCHUNK, NO], f16, kind="ExternalInput").ap()
    b2e_d = nc.dram_tensor("b2e", [1, NO], f32, kind="ExternalInput").ap()
    # outputs (per-core batch slice), free layout = (t, o)
    mem2_d = nc.dram_tensor("mem2rec", [BLOC, T * NO], f32, kind="ExternalOutput").ap()
    spk2_d = nc.dram_tensor("spk2rec", [BLOC, T * NO], f32, kind="ExternalOutput").ap()
    if DEBUG:
        curb_dbg = nc.dram_tensor("curbdbg", [128, NCHUNK * BLOC], f32, kind="ExternalOutput").ap()
        cur2_dbg = nc.dram_tensor("cur2dbg", [128, NGROUP * NO], f32, kind="ExternalOutput").ap()

    with tile.TileContext(nc) as tc:
        with (
            tc.tile_pool(name="xres", bufs=1) as xres,
            tc.tile_pool(name="win", bufs=3) as wpool,
            tc.tile_pool(name="psA", bufs=2, space="PSUM") as psA,
            tc.tile_pool(name="stage", bufs=1) as stage,
            tc.tile_pool(name="dram", bufs=1, space="DRAM") as dram,
            tc.tile_pool(name="mem", bufs=4) as mpool,
            tc.tile_pool(name="g4", bufs=3) as gpool,
            tc.tile_pool(name="psC", bufs=2, space="PSUM") as psC,
            tc.tile_pool(name="psT", bufs=2, space="PSUM") as psT,
        ):
            # ---------------- x resident tiles + small constants up front
            # kt0 x slices and the first W tiles go first so the PE can start
            # within ~8us; the rest streams behind them.
            xh_res = xres.tile([128, KTILES, B], f16, tag="xh")
            xl_res = xres.tile([128, KTILES, B], f16, tag="xl")
            nc.sync.dma_start(xh_res[:, 0:1, :], xth_d[:, 0:1, :])
            nc.sync.dma_start(xl_res[:, 0:1, :], xtl_d[:, 0:1, :])
            KG = 4  # kt per W-tile
            w0l_t = wpool.tile([128, KG, HHALF], f16, tag="w1l")
            w0h_t = wpool.tile([128, KG, HHALF], f16, tag="w1h")
            for j in range(2):
                ksl = slice(j * 2, (j + 1) * 2)
                nc.sync.dma_start(w0l_t[:, ksl, :], w1l_d[:, ksl, 0:HHALF])
                nc.sync.dma_start(w0h_t[:, ksl, :], w1h_d[:, ksl, 0:HHALF])
            for i, (k0, k1) in enumerate([(1, 4), (4, 8), (8, 12), (12, 16),
                                          (16, 20), (20, 24), (24, 28), (28, 32)]):
                nc.sync.dma_start(xh_res[:, k0:k1, :], xth_d[:, k0:k1, :])
                nc.sync.dma_start(xl_res[:, k0:k1, :], xtl_d[:, k0:k1, :])

            b1t = stage.tile([128, NCHUNK], f32, tag="b1t")
            nc.sync.dma_start(b1t[:], b1c_d[:])
            w2f_t = stage.tile([128, NCHUNK, NO], f16, tag="w2f")
            nc.sync.dma_start(w2f_t[:], w2f_d[:])
            b2e_t = stage.tile([1, NO], f32, tag="b2e")
            nc.sync.dma_start(b2e_t[:], b2e_d[:])

            ident = stage.tile([BLOC, BLOC], f32, tag="ident")
            make_identity(nc, ident[:])
            ones_t = stage.tile([1, 128], f32, tag="ones")
            nc.vector.memset(ones_t[:], 1.0)
            biasm1 = stage.tile([128, 1], f32, tag="bm1")
            nc.vector.memset(biasm1[:], -1.0)
            zeros_t = stage.tile([128, NCHUNK * BLOC], f32, tag="zeros")
            nc.vector.memset(zeros_t[:], 0.0)
            z32 = stage.tile([BLOC, NO], f32, tag="z32")
            nc.vector.memset(z32[:], 0.0)

            # ---------------- phase A: two hidden halves, RS pipelined
            partials = []
            rs_outs = []
            for h in range(2):
                hs = slice(h * HHALF, (h + 1) * HHALF)
                ps = [psA.tile([128, HHALF], f32, tag=f"psA{mb}", name=f"ps{h}{mb}")
                      for mb in range(2)]
                for kg in range(KTILES // KG):
                    if h == 0 and kg == 0:
                        wl_t, wh_t = w0l_t, w0h_t
                    else:
                        wl_t = wpool.tile([128, KG, HHALF], f16, tag="w1l")
                        nc.sync.dma_start(wl_t[:], w1l_d[:, kg * KG:(kg + 1) * KG, hs])
                        wh_t = wpool.tile([128, KG, HHALF], f16, tag="w1h")
                        nc.sync.dma_start(wh_t[:], w1h_d[:, kg * KG:(kg + 1) * KG, hs])
                    for i in range(KG):
                        kt = kg * KG + i
                        first = kt == 0
                        last = kt == KTILES - 1
                        for mb in range(2):
                            bs = slice(mb * 128, (mb + 1) * 128)
                            xh_s = xh_res[:, kt, bs]
                            xl_s = xl_res[:, kt, bs]
                            out = ps[mb][:]
                            nc.tensor.matmul(out, xh_s, wl_t[:, i, :],
                                             start=first, stop=False)
                            nc.tensor.matmul(out, xh_s, wh_t[:, i, :],
                                             start=False, stop=False)
                            nc.tensor.matmul(out, xl_s, wh_t[:, i, :],
                                             start=False, stop=last)
                partial = dram.tile([B, HHALF], f32, tag=f"partial{h}")
                for mb in range(2):
                    cs = stage.tile([128, HHALF], f32, tag=f"curp{mb}", name=f"cs{h}{mb}")
                    nc.scalar.activation(cs[:], ps[mb][:],
                                         mybir.ActivationFunctionType.Copy,
                                         scale=1.0 / W1SCALE)
                    for q in range(4):
                        qs = slice(q * (HHALF // 4), (q + 1) * (HHALF // 4))
                        nc.sync.dma_start(partial[mb * 128:(mb + 1) * 128, qs],
                                          cs[:, qs])
                rs_out = dram.tile([BLOC, HHALF], f32, tag=f"rsout{h}")
                if sim:
                    nc.sync.dma_start(rs_out[:], partial[0:BLOC, :])
                else:
                    nc.gpsimd.collective_compute(
                        "ReduceScatter",
                        mybir.AluOpType.add,
                        replica_groups=[list(range(N_CORES))],
                        ins=[partial.opt()],
                        outs=[rs_out.opt()],
                    )
                partials.append(partial)
                rs_outs.append(rs_out)

            # ---------------- transpose to scan layout + fold b1
            # curb[p, c*32 + b] = cur1[b, c*128 + p] + b1[c*128 + p]
            curb = stage.tile([128, NCHUNK * BLOC], f32, tag="curb")
            for h in range(2):
                rsb = stage.tile([BLOC, HHALF], f32, tag="rsb", name=f"rsb{h}")
                nc.sync.dma_start(rsb[:], rs_outs[h][:])
                for cc in range(NCHUNK // 2):
                    c = h * (NCHUNK // 2) + cc
                    pt = psT.tile([128, BLOC], f32, tag="pst")
                    nc.tensor.transpose(pt[:], rsb[:, cc * 128:(cc + 1) * 128], ident[:])
                    nc.scalar.activation(
                        curb[:, c * BLOC:(c + 1) * BLOC], pt[:],
                        mybir.ActivationFunctionType.Identity,
                        bias=b1t[:, c:c + 1], scale=1.0,
                    )

            # ---------------- phase B/C/D: fused scan
            cur2buf = stage.tile([128, NGROUP * NO], f32, tag="cur2buf")
            # batch-major layer-2 buffers: [32, (g, s, o)] == output layout
            cur2r = stage.tile([BLOC, T * NO], f32, tag="cur2r")
            cur2r_v = cur2r[:].rearrange("p (g s o) -> p g s o", s=G, o=NO)
            mem2r = stage.tile([BLOC, T * NO], f32, tag="mem2r")

            def fold_and_phase_d(g0, g1):
                """Move cur2buf groups [g0,g1) to batch-major cur2r, then run
                the layer-2 LIF steps for those groups."""
                for sl in range(G):
                    nc.sync.dma_start(
                        cur2r_v[:, g0:g1, sl, :],
                        cur2buf[sl * BLOC:(sl + 1) * BLOC,
                                g0 * NO:g1 * NO].rearrange("p (g o) -> p g o", o=NO),
                    )
                for t2 in range(g0 * G, g1 * G):
                    in0 = z32[:] if t2 == 0 else mem2r[:, (t2 - 1) * NO:t2 * NO]
                    nc.vector._custom_dve(
                        LIF,
                        out=mem2r[:, t2 * NO:(t2 + 1) * NO],
                        in0=in0,
                        in1=cur2r[:, t2 * NO:(t2 + 1) * NO],
                        s0=BETA,
                    )

            SB = 12   # groups per phase-D superbatch (multiple of QB)
            QB = 4    # groups per PSUM tile / ACT copy batch
            next_d = 0
            mg_prev = None
            pc4 = None
            for gi in range(NGROUP):
                mg = mpool.tile([128, G, NCHUNK * BLOC], f32, tag="mg")
                for sl in range(G):
                    if gi == 0 and sl == 0:
                        in0 = zeros_t[:]
                    elif sl == 0:
                        in0 = mg_prev[:, G - 1, :]
                    else:
                        in0 = mg[:, sl - 1, :]
                    nc.vector._custom_dve(LIF, out=mg[:, sl, :], in0=in0,
                                          in1=curb[:], s0=BETA)
                mg_prev = mg
                # batched sign for the whole group: gt[p, c, s*32+b] = sign(m-1)
                gt = gpool.tile([128, NCHUNK, G * BLOC], f16, tag="gt")
                nc.scalar.activation(
                    gt[:].rearrange("p c (s b) -> p s c b", b=BLOC),
                    mg[:].rearrange("p s (c b) -> p s c b", b=BLOC),
                    mybir.ActivationFunctionType.Sign, bias=biasm1[:], scale=1.0,
                )
                # phase C: cur2 for the group (single fp16 product per chunk);
                # one PSUM tile serves QB groups so the ACT copy happens once
                # per quad and never blocks the sign stream.
                gq, gr = divmod(gi, QB)
                if gr == 0:
                    pc4 = psC.tile([128, QB * NO], f32, tag="psc")
                out_sl = pc4[:, gr * NO:(gr + 1) * NO]
                for c in range(NCHUNK):
                    nc.tensor.matmul(out_sl, gt[:, c, :], w2f_t[:, c, :],
                                     start=(c == 0), stop=False)
                nc.tensor.matmul(out_sl, ones_t[:], b2e_t[:], start=False, stop=True)
                if gr == QB - 1 or gi == NGROUP - 1:
                    nc.scalar.activation(
                        cur2buf[:, gq * QB * NO:(gi + 1) * NO], pc4[:, 0:(gr + 1) * NO],
                        mybir.ActivationFunctionType.Copy,
                    )
                # fold+phase-D once deps are a few groups stale (no DVE stall)
                if gi >= next_d + SB + 2:
                    fold_and_phase_d(next_d, next_d + SB)
                    next_d += SB
            fold_and_phase_d(next_d, NGROUP)
            if DEBUG:
                nc.sync.dma_start(curb_dbg[:], curb[:])
                nc.sync.dma_start(cur2_dbg[:], cur2buf[:])

            spk2 = stage.tile([BLOC, T * NO], f32, tag="spk2")
            nc.vector.tensor_scalar(spk2[:], mem2r[:], 1.0, None, mybir.AluOpType.is_gt)
            nc.sync.dma_start(mem2_d[:], mem2r[:])
            nc.sync.dma_start(spk2_d[:], spk2[:])

    nc.compile()
    _PROGRAMS[sim] = (nc, LIF)
    return _PROGRAMS[sim]


# ---------------------------------------------------------------- host prep
def _prep_inputs(x, W1, b1, W2, b2):
    f32 = np.float32
    x_flat = np.ascontiguousarray(x.reshape(B, -1).astype(f32, copy=False))  # [256, 32000]
    xT = np.zeros((KPAD, B), f32)
    xT[:NIN] = x_flat.T
    xTh = xT.astype(np.float16)
    xTl = (xT - xTh.astype(f32)).astype(np.float16)
    w1T = np.zeros((KPAD, HPAD), f32)
    w1T[:NIN, :NH] = W1.astype(f32, copy=False).T * W1SCALE
    w1Th = w1T.astype(np.float16)
    w1Tl = (w1T - w1Th.astype(f32)).astype(np.float16)
    b1p = np.full(HPAD, -10.0, f32)
    b1p[:NH] = b1
    b1c = np.ascontiguousarray(b1p.reshape(NCHUNK, 128).T)          # [128, 8]
    W2e = np.zeros((HPAD, NO), f32)
    W2e[:NH] = 0.5 * W2.astype(f32, copy=False).T
    w2stack = np.ascontiguousarray(W2e.reshape(NCHUNK, 128, NO).transpose(1, 0, 2))  # [128,8,5]
    w2f = w2stack.astype(np.float16)
    b2e = (b2.astype(f32) + 0.5 * W2.astype(f32).sum(axis=1)).reshape(1, NO).astype(f32)

    in_maps = []
    for c in range(N_CORES):
        ksl = slice(c * KC, (c + 1) * KC)
        # partition-major relayouts: [128, KTILES, cols]
        xh_r = np.ascontiguousarray(xTh[ksl].reshape(KTILES, 128, B).transpose(1, 0, 2))
        xl_r = np.ascontiguousarray(xTl[ksl].reshape(KTILES, 128, B).transpose(1, 0, 2))
        wh_r = np.ascontiguousarray(w1Th[ksl].reshape(KTILES, 128, HPAD).transpose(1, 0, 2))
        wl_r = np.ascontiguousarray(w1Tl[ksl].reshape(KTILES, 128, HPAD).transpose(1, 0, 2))
        in_maps.append({
            "xth": xh_r,
            "xtl": xl_r,
            "w1h": wh_r,
            "w1l": wl_r,
            "b1c": b1c,
            "w2f": w2f,
            "b2e": b2e,
        })
    return in_maps


def _gather(results):
    spk_parts, mem_parts = [], []
    for r in results:
        mem_parts.append(r["mem2rec"].reshape(BLOC, T, NO).transpose(1, 0, 2))
        spk_parts.append(r["spk2rec"].reshape(BLOC, T, NO).transpose(1, 0, 2))
    mem2 = np.concatenate(mem_parts, axis=1).astype(np.float32)  # [200, 256, 5]
    spk2 = np.concatenate(spk_parts, axis=1).astype(np.float32)
    return spk2, mem2


def run_raw(inputs, **kwargs):
    """Build+run; returns BassKernelResults (for profiling from test.py)."""
    from concourse.bass_utils import run_bass_kernel_spmd

    nc, _ = _build_program()
    in_maps = _prep_inputs(**inputs)
    return run_bass_kernel_spmd(nc, in_maps, core_ids=list(range(N_CORES)), **kwargs)


def kernel(x, W1, b1, W2, b2):
    res = run_raw(dict(x=x, W1=W1, b1=b1, W2=W2, b2=b2))
    return _gather(res.results)


if __name__ == "__main__":
    rng = np.random.default_rng(0)
    ins = {
        "x": rng.standard_normal((B, 2, 80, 200)).astype(np.float32),
        "W1": rng.uniform(-1, 1, (NH, NIN)).astype(np.float32) / np.sqrt(NIN),
        "b1": rng.uniform(-1, 1, NH).astype(np.float32) / np.sqrt(NIN),
        "W2": rng.uniform(-1, 1, (NO, NH)).astype(np.float32) / np.sqrt(NH),
        "b2": rng.uniform(-1, 1, NO).astype(np.float32) / np.sqrt(NH),
    }
    spk2, mem2 = kernel(**ins)
    print("shapes:", spk2.shape, mem2.shape, spk2.dtype, mem2.dtype)
    print("spk2 mean:", spk2.mean(), "mem2 std:", mem2.std())
